# revision 1
# baseline (speedup 1.0000x reference)
"""Trainium2 Bass kernel for nn_EncoderMemNN_14929306321427 (MemNN encoder).

Math (see reference.py): story (M=256, B=16, S=64) token ids; C (4, V, 128)
embedding tables. Per hop h: m_A = sum_S C[h][s], prob = softmax_M(m_A @ u),
m_C = sum_S C[h+1][s], u += prob @ m_C. u starts at 0, so hop-0's softmax is
uniform: C[0] is never needed and u after hop 0 is mean_M(E1).

Strategy: data-parallel over batch (2 rows/core, 8 cores, no collectives).
Host fuses tables 1..3 into ccat[V+1, 384] fp16 (row V = 0) so each token is
ONE 768B dma_gather row. dma_gather indices are int16, so tokens are split at
32768: call A gathers low tokens from the table base, call B gathers high
tokens from a +32768 row view; slots not owned by a call point at an all-zero
row (PAD row 0 / appended row V), which adds 0 to the sum. Tokens are sorted
within each sentence and sentences are nlow-balanced across groups so the two
calls cover disjoint near-minimal slot ranges (~6% filler). The sentence-sum
runs on the PE as identity-matmul accumulation into PSUM (fp32-exact), then a
tiny PE/ACT/DVE attention pipeline computes the 3 hops.
"""

import numpy as np

HOPS = 3
V = 50257
D = 128
M = 256
B = 16
S = 64
NCORES = 8
BL = B // NCORES            # batch rows per core
NS = BL * M                 # sentences per core
P = 128
NG = NS // P                # sentence groups of 128
DCAT = HOPS * D             # 384 = fused row [C1|C2|C3]
NEG = -1e30
VSPLIT = 32768
ZHIGH = V - VSPLIT          # index of appended zero row within the high view

_CACHE = {}


def _consts():
    ident = np.eye(P, dtype=np.float32)
    i2 = np.eye(2, dtype=np.float32)
    identg = np.eye(P, dtype=np.float16)
    return {"ident": ident, "i2": i2, "identg": identg}


def build(KA, KB, do_compile=True):
    """KA/KB: per-group slot counts for the low/high gather calls."""
    from concourse import bacc, mybir, tile

    f32 = mybir.dt.float32
    f16 = mybir.dt.float16
    i16 = mybir.dt.int16
    Alu = mybir.AluOpType
    Act = mybir.ActivationFunctionType
    Ax = mybir.AxisListType

    nc = bacc.Bacc(num_swdge_queues=2)
    ccat_d = nc.declare_dram_parameter("ccat", [V + 1, DCAT], f16, isOutput=False)
    idx_d = {}
    for g in range(NG):
        idx_d[g, "a"] = nc.declare_dram_parameter(
            f"idxa{g}", [P, P * KA[g] // 16], i16, isOutput=False)
        idx_d[g, "b"] = nc.declare_dram_parameter(
            f"idxb{g}", [P, P * KB[g] // 16], i16, isOutput=False)
    ident_d = nc.declare_dram_parameter("ident", [P, P], f32, isOutput=False)
    identg_d = nc.declare_dram_parameter("identg", [P, P], f16, isOutput=False)
    i2_d = nc.declare_dram_parameter("i2", [2, 2], f32, isOutput=False)
    sel_d = nc.declare_dram_parameter("sel", [P, NG * 2], f32, isOutput=False)
    mneg_d = nc.declare_dram_parameter("mneg", [BL, BL * M], f32, isOutput=False)
    out_d = nc.declare_dram_parameter("out", [BL, D], f32, isOutput=True)

    with tile.TileContext(nc) as tc:
        with (
            tc.tile_pool(name="const", bufs=1) as cpool,
            tc.tile_pool(name="gather", bufs=2) as gpool,
            tc.tile_pool(name="work", bufs=2) as wpool,
            tc.tile_pool(name="ps_e", bufs=2, space="PSUM") as ps_e,
            tc.tile_pool(name="ps_t", bufs=2, space="PSUM") as ps_t,
            tc.tile_pool(name="ps_col", bufs=1, space="PSUM") as ps_col,
            tc.tile_pool(name="ps_mm", bufs=1, space="PSUM") as ps_mm,
        ):
            idx_sb = {}
            for g in range(NG):
                for h in ("a", "b"):
                    t = cpool.tile(list(idx_d[g, h].shape), i16, tag=f"idx{h}{g}")
                    nc.sync.dma_start(out=t[:], in_=idx_d[g, h][:])
                    idx_sb[g, h] = t
            ident = cpool.tile([P, P], f32)
            nc.sync.dma_start(out=ident[:], in_=ident_d[:])
            identg = cpool.tile([P, P], f16)
            nc.sync.dma_start(out=identg[:], in_=identg_d[:])
            i2 = cpool.tile([2, 2], f32)
            nc.sync.dma_start(out=i2[:], in_=i2_d[:])
            sel = cpool.tile([P, NG * 2], f32)
            nc.sync.dma_start(out=sel[:], in_=sel_d[:])
            mneg = cpool.tile([BL, BL * M], f32)
            nc.sync.dma_start(out=mneg[:], in_=mneg_d[:])

            # ---- gather + sentence-sum: E_all[p, g*DCAT+d] = sum_S ccat[tok]
            E_all = cpool.tile([P, NG * DCAT], f32)
            for g in range(NG):
                gta = gpool.tile([P, KA[g], DCAT], f16, tag="gta")
                nc.gpsimd.dma_gather(
                    out_ap=gta[:], in_ap=ccat_d[:], idxs_ap=idx_sb[g, "a"][:],
                    num_idxs=P * KA[g], num_idxs_reg=P * KA[g],
                    elem_size=DCAT, single_packet=False,
                )
                gtb = gpool.tile([P, KB[g], DCAT], f16, tag="gtb")
                nc.gpsimd.dma_gather(
                    out_ap=gtb[:], in_ap=ccat_d[VSPLIT:, :], idxs_ap=idx_sb[g, "b"][:],
                    num_idxs=P * KB[g], num_idxs_reg=P * KB[g],
                    elem_size=DCAT, single_packet=False, queue_num=1,
                )
                eps = ps_e.tile([P, DCAT], f32, tag="eacc")
                tot = KA[g] + KB[g]
                nmm = 0
                for gt, kk in ((gta, KA[g]), (gtb, KB[g])):
                    for r in range(kk):
                        nc.tensor.matmul(
                            out=eps[:], lhsT=identg[:], rhs=gt[:, r, :],
                            start=(nmm == 0), stop=(nmm == tot - 1),
                        )
                        nmm += 1
                nc.vector.tensor_copy(out=E_all[:, g * DCAT:(g + 1) * DCAT], in_=eps[:])

            # transposed E1/E2 for the logits matmuls (filled per group):
            # F_t[:, g*P:(g+1)*P] = (E_t block of group g).T   [d, sentence]
            F1 = cpool.tile([P, NS], f32)
            F2 = cpool.tile([P, NS], f32)
            us = ps_mm.tile([BL, DCAT], f32, tag="usum")
            for g in range(NG):
                for t, F in ((0, F1), (1, F2)):
                    tp = ps_t.tile([P, P], f32, tag="tp")
                    nc.tensor.transpose(
                        out=tp[:],
                        in_=E_all[:, g * DCAT + t * D: g * DCAT + t * D + D],
                        identity=ident[:],
                    )
                    nc.scalar.copy(out=F[:, g * P:(g + 1) * P], in_=tp[:])
                # hop 0: u = mean_M E1[b] (softmax of zero logits is uniform);
                # sel col b marks this group's sentences owned by batch row b
                nc.tensor.matmul(
                    out=us[:], lhsT=sel[:, g * 2:(g + 1) * 2],
                    rhs=E_all[:, g * DCAT:(g + 1) * DCAT],
                    start=(g == 0), stop=(g == NG - 1),
                )
            u = wpool.tile([BL, D], f32, tag="u0")
            nc.scalar.activation(
                out=u[:], in_=us[0:BL, 0:D], func=Act.Copy, scale=1.0 / M
            )

            # ---- hops 1..2
            for hop in (1, 2):
                F = F1 if hop == 1 else F2
                # u as columns: uc[d, b] = u[b, d]
                uc_ps = ps_col.tile([P, BL], f32, tag="colT")
                nc.tensor.matmul(out=uc_ps[:], lhsT=u[:], rhs=i2[:], start=True, stop=True)
                uc = wpool.tile([P, BL], f32, tag="uc")
                nc.scalar.copy(out=uc[:], in_=uc_ps[:])
                # logits[b, b'*M+m] = sum_d u[b,d] * E_hop[b',m,d]; mask kills b!=b'
                lg_ps = ps_mm.tile([BL, NS], f32, tag="lg")
                nc.tensor.matmul(out=lg_ps[:], lhsT=uc[:], rhs=F[:], start=True, stop=True)
                lgm = wpool.tile([BL, NS], f32, tag="lgm")
                nc.vector.scalar_tensor_tensor(
                    out=lgm[:], in0=lg_ps[:], scalar=1.0, in1=mneg[:],
                    op0=Alu.mult, op1=Alu.add,
                )
                nmax = wpool.tile([BL, 1], f32, tag="nmax")
                nc.vector.tensor_reduce(
                    out=nmax[:], in_=lgm[:], axis=Ax.X, op=Alu.max, negate=True
                )
                pe = wpool.tile([BL, NS], f32, tag="pe")
                den = wpool.tile([BL, 1], f32, tag="den")
                nc.scalar.activation(
                    out=pe[:], in_=lgm[:], func=Act.Exp, bias=nmax[:], scale=1.0,
                    accum_out=den[:],
                )
                rden = wpool.tile([BL, 1], f32, tag="rden")
                nc.vector.reciprocal(out=rden[:], in_=den[:])
                # o[b, d] = sum_m pe[b, m] * E_{hop+1}[b, m, d]  (normalized below)
                o_ps = ps_mm.tile([BL, D], f32, tag="o")
                for g in range(NG):
                    pt_ps = ps_col.tile([P, BL], f32, tag="colT")
                    nc.tensor.matmul(
                        out=pt_ps[:], lhsT=pe[:, g * P:(g + 1) * P], rhs=i2[:],
                        start=True, stop=True,
                    )
                    ptsb = wpool.tile([P, BL], f32, tag="ptsb")
                    nc.scalar.copy(out=ptsb[:], in_=pt_ps[:])
                    nc.tensor.matmul(
                        out=o_ps[:], lhsT=ptsb[:],
                        rhs=E_all[:, g * DCAT + hop * D: g * DCAT + hop * D + D],
                        start=(g == 0), stop=(g == NG - 1),
                    )
                # u <- u + o / den
                u2 = wpool.tile([BL, D], f32, tag=f"u{hop}")
                nc.vector.scalar_tensor_tensor(
                    out=u2[:], in0=o_ps[:], scalar=rden[:], in1=u[:],
                    op0=Alu.mult, op1=Alu.add,
                )
                u = u2

            nc.sync.dma_start(out=out_d[:], in_=u[:])
    if do_compile:
        nc.compile()
    return nc


def _wrap16(idx):
    """flat [n] int16 -> SBUF layout [128, n//16]: value i at [i%16, i//16],
    replicated to the 8 16-partition groups the Q7 cores read."""
    n = idx.shape[0]
    w = np.zeros((16, n // 16), np.int16)
    w[np.arange(n) % 16, np.arange(n) // 16] = idx
    return np.tile(w, (8, 1))


def prep_inputs(story, C):
    """Host-side: fused fp16 table, sorted/balanced per-core index layouts."""
    story = np.asarray(story)
    C = np.asarray(C, dtype=np.float32)
    s = story.transpose(1, 0, 2).astype(np.int32)       # (B, M, S)
    ccat = np.zeros((V + 1, DCAT), np.float16)
    ccat[:V] = np.concatenate([C[1], C[2], C[3]], axis=1).astype(np.float16)

    # per core: sort tokens in each sentence (low vocab first) and balance
    # sentences across the NG groups by nlow, mixing batch rows freely; the
    # uploaded sel/mneg tensors encode each sentence's batch-row ownership
    sorted_toks = []                                     # [core][g] -> (P, S)
    nlows = []                                           # [core][g] -> (P,)
    owners = []                                          # [core][g] -> (P,) batch row
    for i in range(NCORES):
        blk = s[i * BL:(i + 1) * BL].reshape(NS, S)      # (NS, S)
        own = np.repeat(np.arange(BL), M)                # (NS,)
        nlow = (blk < VSPLIT).sum(1)
        order = np.argsort(nlow, kind="stable")
        st_c, nl_c, ow_c = [], [], []
        for q in ((0, 3, 1, 2) if NG == 4 else range(NG)):
            pick = order[q * P:(q + 1) * P]
            st_c.append(np.sort(blk[pick], axis=1))
            nl_c.append(nlow[pick])
            ow_c.append(own[pick])
        sorted_toks.append(st_c)
        nlows.append(nl_c)
        owners.append(ow_c)

    KA = tuple(int(max(nlows[i][g].max() for i in range(NCORES))) for g in range(NG))
    KB = tuple(S - int(min(nlows[i][g].min() for i in range(NCORES))) for g in range(NG))

    consts = _consts()
    in_maps = []
    for i in range(NCORES):
        m = {"ccat": ccat, **consts}
        sel = np.zeros((P, NG * 2), np.float32)
        mneg = np.full((BL, BL * M), NEG, np.float32)
        for g in range(NG):
            sel[np.arange(P), g * 2 + owners[i][g]] = 1.0
            mneg[owners[i][g], g * P + np.arange(P)] = 0.0
        m["sel"] = sel
        m["mneg"] = mneg
        for g in range(NG):
            toks = sorted_toks[i][g]                     # (P, S) sorted
            nlow = nlows[i][g]                           # (P,)
            ka, kb = KA[g], KB[g]
            ks = np.arange(ka)[:, None]                  # slot k -> row k*128+p
            low = np.where(ks < nlow[None, :], toks.T[:ka], 0).astype(np.int16)
            m[f"idxa{g}"] = _wrap16(low.reshape(-1))
            k0 = S - kb
            ksb = (k0 + np.arange(kb))[:, None]
            high = np.where(
                ksb >= nlow[None, :],
                toks.T[k0:].astype(np.int64) - VSPLIT,
                ZHIGH,
            ).astype(np.int16)
            m[f"idxb{g}"] = _wrap16(high.reshape(-1))
        in_maps.append(m)
    return in_maps, KA, KB


def run(in_maps, KA, KB, trace=False, **kwargs):
    from concourse.bass_utils import run_bass_kernel_spmd

    key = (KA, KB)
    if key not in _CACHE:
        _CACHE[key] = build(KA, KB)
    nc = _CACHE[key]
    res = run_bass_kernel_spmd(
        nc, in_maps, core_ids=list(range(NCORES)), trace=trace, **kwargs
    )
    out = np.concatenate([r["out"] for r in res.results], axis=0)
    return out, res


def kernel(story, C):
    in_maps, KA, KB = prep_inputs(story, C)
    out, _ = run(in_maps, KA, KB)
    return out.astype(np.float32)



# revision 2
# speedup vs baseline: 1.3573x; 1.3573x over previous
"""Trainium2 Bass kernel for nn_EncoderMemNN_14929306321427 (MemNN encoder).

Math (see reference.py): story (M=256, B=16, S=64) token ids; C (4, V, 128)
embedding tables. Per hop h: m_A = sum_S C[h][s], prob = softmax_M(m_A @ u),
m_C = sum_S C[h+1][s], u += prob @ m_C. u starts at 0, so hop-0's softmax is
uniform: C[0] is never needed and u after hop 0 is mean_M(E1).

Strategy: data-parallel over batch (2 rows/core, 8 cores, no collectives).
Host fuses tables 1..3 into ccat[V+1, 384] fp16 (row V = 0) so each token is
ONE 768B dma_gather row. dma_gather indices are int16, so tokens are split at
32768: low tokens gather from the table base, high tokens from a +32768 row
view; slots not owned by a call point at an all-zero row (PAD row 0 /
appended row V), which adds 0 to the sum. Tokens are sorted within each
sentence and sentences are nlow-balanced across groups so the two sections
cover disjoint near-minimal slot ranges (~10% filler). The sentence-sum runs
on the PE as identity-matmul accumulation into PSUM (fp32-exact), then a tiny
PE/ACT/DVE attention pipeline computes the 3 hops.

Perf: SWDGE descriptor generation is the bottleneck (~7.9 ns/desc), and each
SWDGE queue is served by a single pair of the 8 Q7 cores (q7 kernel:
cpu_id/2 == queue_num). So each group's gather is split into ~equal slot
ranges across all 4 SWDGE queues, engaging all 8 Q7 cores in parallel.
Queue-0 gathers block the Pool engine for their whole generation (observed;
queues 1-3 don't), so each group's queue-0 call is dispatched last.
"""

import numpy as np

HOPS = 3
V = 50257
D = 128
M = 256
B = 16
S = 64
NCORES = 8
BL = B // NCORES            # batch rows per core
NS = BL * M                 # sentences per core
P = 128
NG = NS // P                # sentence groups of 128
DCAT = HOPS * D             # 384 = fused row [C1|C2|C3]
NEG = -1e30
VSPLIT = 32768
ZHIGH = V - VSPLIT          # index of appended zero row within the high view
NQ = 4                      # SWDGE queues (all 8 Q7 cores)

_CACHE = {}


def _consts():
    ident = np.eye(P, dtype=np.float32)
    i2 = np.eye(2, dtype=np.float32)
    identg = np.eye(P, dtype=np.float16)
    return {"ident": ident, "i2": i2, "identg": identg}


def _plan(ka, kb):
    """Split the concatenated [A slots | B slots] list of one group into NQ
    near-equal contiguous ranges, one per SWDGE queue; a range crossing the
    A/B boundary becomes two calls on the same queue. Queue 0 is dispatched
    last (its generation blocks the Pool engine)."""
    tot = ka + kb
    bounds = [round(tot * i / NQ) for i in range(NQ + 1)]
    calls = []                       # (section, slot_off, slot_cnt, queue)
    for i in range(NQ):
        q = (i + 1) % NQ             # ranges -> queues 1,2,3,0
        s, e = bounds[i], bounds[i + 1]
        if s < ka and min(e, ka) > s:
            calls.append(("a", s, min(e, ka) - s, q))
        if e > ka and e > max(s, ka):
            calls.append(("b", max(s, ka) - ka, e - max(s, ka), q))
    calls.sort(key=lambda c: c[3] == 0)   # stable: queues 1,2,3 first, 0 last
    return calls


def build(KA, KB, do_compile=True):
    """KA/KB: per-group slot counts for the low/high gather sections."""
    from concourse import bacc, mybir, tile

    f32 = mybir.dt.float32
    f16 = mybir.dt.float16
    i16 = mybir.dt.int16
    Alu = mybir.AluOpType
    Act = mybir.ActivationFunctionType
    Ax = mybir.AxisListType

    nc = bacc.Bacc(num_swdge_queues=NQ)
    ccat_d = nc.declare_dram_parameter("ccat", [V + 1, DCAT], f16, isOutput=False)
    # all groups' wrapped idx arrays fused into one tensor: [P, 8*(KA+KB) tot]
    NIDX = 8 * (sum(KA) + sum(KB))
    idx_d = nc.declare_dram_parameter("idx", [P, NIDX], i16, isOutput=False)
    # fused f32 consts: [ident | sel | i2pad] = [P, P + NG*2 + 2]
    cst_d = nc.declare_dram_parameter("cst", [P, P + NG * 2 + 2], f32, isOutput=False)
    identg_d = nc.declare_dram_parameter("identg", [P, P], f16, isOutput=False)
    mneg_d = nc.declare_dram_parameter("mneg", [BL, BL * M], f32, isOutput=False)
    out_d = nc.declare_dram_parameter("out", [BL, D], f32, isOutput=True)

    # column offset of group g's section ('a'/'b') within idx_d
    idx_off = {}
    off = 0
    for g in range(NG):
        idx_off[g, "a"] = off
        off += 8 * KA[g]
        idx_off[g, "b"] = off
        off += 8 * KB[g]

    with tile.TileContext(nc) as tc:
        with (
            tc.tile_pool(name="const", bufs=1) as cpool,
            tc.tile_pool(name="gather", bufs=2) as gpool,
            tc.tile_pool(name="work", bufs=2) as wpool,
            tc.tile_pool(name="ps_e", bufs=2, space="PSUM") as ps_e,
            tc.tile_pool(name="ps_t", bufs=2, space="PSUM") as ps_t,
            tc.tile_pool(name="ps_col", bufs=1, space="PSUM") as ps_col,
            tc.tile_pool(name="ps_mm", bufs=1, space="PSUM") as ps_mm,
        ):
            idx_sb = cpool.tile([P, NIDX], i16)
            nc.sync.dma_start(out=idx_sb[:], in_=idx_d[:])
            cst = cpool.tile([P, P + NG * 2 + 2], f32)
            nc.sync.dma_start(out=cst[:], in_=cst_d[:])
            identg = cpool.tile([P, P], f16)
            nc.sync.dma_start(out=identg[:], in_=identg_d[:])
            mneg = cpool.tile([BL, BL * M], f32)
            nc.sync.dma_start(out=mneg[:], in_=mneg_d[:])
            ident = cst[:, 0:P]
            sel = cst[:, P:P + NG * 2]
            i2 = cst[0:2, P + NG * 2:P + NG * 2 + 2]

            # ---- gather + sentence-sum: E_all[p, g*DCAT+d] = sum_S ccat[tok]
            E_all = cpool.tile([P, NG * DCAT], f32)
            for g in range(NG):
                tiles = []
                for j, (sec, soff, cnt, q) in enumerate(_plan(KA[g], KB[g])):
                    ct = gpool.tile([P, cnt, DCAT], f16, tag=f"gt{j}")
                    src = ccat_d[:] if sec == "a" else ccat_d[VSPLIT:, :]
                    col = idx_off[g, sec] + soff * 8
                    nc.gpsimd.dma_gather(
                        out_ap=ct[:], in_ap=src,
                        idxs_ap=idx_sb[:, col:col + cnt * 8],
                        num_idxs=P * cnt, num_idxs_reg=P * cnt,
                        elem_size=DCAT, single_packet=False, queue_num=q,
                    )
                    tiles.append((ct, cnt))
                eps = ps_e.tile([P, DCAT], f32, tag="eacc")
                tot = KA[g] + KB[g]
                nmm = 0
                for ct, cnt in tiles:
                    for r in range(cnt):
                        nc.tensor.matmul(
                            out=eps[:], lhsT=identg[:], rhs=ct[:, r, :],
                            start=(nmm == 0), stop=(nmm == tot - 1),
                        )
                        nmm += 1
                nc.vector.tensor_copy(out=E_all[:, g * DCAT:(g + 1) * DCAT], in_=eps[:])

            # transposed E1/E2 for the logits matmuls (filled per group):
            # F_t[:, g*P:(g+1)*P] = (E_t block of group g).T   [d, sentence]
            F1 = cpool.tile([P, NS], f32)
            F2 = cpool.tile([P, NS], f32)
            us = ps_mm.tile([BL, DCAT], f32, tag="usum")
            for g in range(NG):
                for t, F in ((0, F1), (1, F2)):
                    tp = ps_t.tile([P, P], f32, tag="tp")
                    nc.tensor.transpose(
                        out=tp[:],
                        in_=E_all[:, g * DCAT + t * D: g * DCAT + t * D + D],
                        identity=ident,
                    )
                    nc.scalar.copy(out=F[:, g * P:(g + 1) * P], in_=tp[:])
                # hop 0: u = mean_M E1[b] (softmax of zero logits is uniform);
                # sel col b marks this group's sentences owned by batch row b
                nc.tensor.matmul(
                    out=us[:], lhsT=sel[:, g * 2:(g + 1) * 2],
                    rhs=E_all[:, g * DCAT:(g + 1) * DCAT],
                    start=(g == 0), stop=(g == NG - 1),
                )
            u = wpool.tile([BL, D], f32, tag="u0")
            nc.scalar.activation(
                out=u[:], in_=us[0:BL, 0:D], func=Act.Copy, scale=1.0 / M
            )

            # ---- hops 1..2
            for hop in (1, 2):
                F = F1 if hop == 1 else F2
                # u as columns: uc[d, b] = u[b, d]
                uc_ps = ps_col.tile([P, BL], f32, tag="colT")
                nc.tensor.matmul(out=uc_ps[:], lhsT=u[:], rhs=i2, start=True, stop=True)
                uc = wpool.tile([P, BL], f32, tag="uc")
                nc.scalar.copy(out=uc[:], in_=uc_ps[:])
                # logits[b, b'*M+m] = sum_d u[b,d] * E_hop[b',m,d]; mask kills b!=b'
                lg_ps = ps_mm.tile([BL, NS], f32, tag="lg")
                nc.tensor.matmul(out=lg_ps[:], lhsT=uc[:], rhs=F[:], start=True, stop=True)
                lgm = wpool.tile([BL, NS], f32, tag="lgm")
                nc.vector.scalar_tensor_tensor(
                    out=lgm[:], in0=lg_ps[:], scalar=1.0, in1=mneg[:],
                    op0=Alu.mult, op1=Alu.add,
                )
                nmax = wpool.tile([BL, 1], f32, tag="nmax")
                nc.vector.tensor_reduce(
                    out=nmax[:], in_=lgm[:], axis=Ax.X, op=Alu.max, negate=True
                )
                pe = wpool.tile([BL, NS], f32, tag="pe")
                den = wpool.tile([BL, 1], f32, tag="den")
                nc.scalar.activation(
                    out=pe[:], in_=lgm[:], func=Act.Exp, bias=nmax[:], scale=1.0,
                    accum_out=den[:],
                )
                rden = wpool.tile([BL, 1], f32, tag="rden")
                nc.vector.reciprocal(out=rden[:], in_=den[:])
                # o[b, d] = sum_m pe[b, m] * E_{hop+1}[b, m, d]  (normalized below)
                o_ps = ps_mm.tile([BL, D], f32, tag="o")
                for g in range(NG):
                    pt_ps = ps_col.tile([P, BL], f32, tag="colT")
                    nc.tensor.matmul(
                        out=pt_ps[:], lhsT=pe[:, g * P:(g + 1) * P], rhs=i2,
                        start=True, stop=True,
                    )
                    ptsb = wpool.tile([P, BL], f32, tag="ptsb")
                    nc.scalar.copy(out=ptsb[:], in_=pt_ps[:])
                    nc.tensor.matmul(
                        out=o_ps[:], lhsT=ptsb[:],
                        rhs=E_all[:, g * DCAT + hop * D: g * DCAT + hop * D + D],
                        start=(g == 0), stop=(g == NG - 1),
                    )
                # u <- u + o / den
                u2 = wpool.tile([BL, D], f32, tag=f"u{hop}")
                nc.vector.scalar_tensor_tensor(
                    out=u2[:], in0=o_ps[:], scalar=rden[:], in1=u[:],
                    op0=Alu.mult, op1=Alu.add,
                )
                u = u2

            nc.sync.dma_start(out=out_d[:], in_=u[:])
    if do_compile:
        nc.compile()
    return nc


def _wrap16(idx):
    """flat [n] int16 -> SBUF layout [128, n//16]: value i at [i%16, i//16],
    replicated to the 8 16-partition groups the Q7 cores read."""
    n = idx.shape[0]
    w = np.zeros((16, n // 16), np.int16)
    w[np.arange(n) % 16, np.arange(n) // 16] = idx
    return np.tile(w, (8, 1))


def prep_inputs(story, C):
    """Host-side: fused fp16 table, sorted/balanced per-core index layouts."""
    story = np.asarray(story)
    C = np.asarray(C, dtype=np.float32)
    s = story.transpose(1, 0, 2).astype(np.int32)       # (B, M, S)
    ccat = np.zeros((V + 1, DCAT), np.float16)
    ccat[:V] = np.concatenate([C[1], C[2], C[3]], axis=1).astype(np.float16)

    # per core: sort tokens in each sentence (low vocab first) and balance
    # sentences across the NG groups by nlow, mixing batch rows freely; the
    # uploaded sel/mneg tensors encode each sentence's batch-row ownership
    sorted_toks = []                                     # [core][g] -> (P, S)
    nlows = []                                           # [core][g] -> (P,)
    owners = []                                          # [core][g] -> (P,) batch row
    for i in range(NCORES):
        blk = s[i * BL:(i + 1) * BL].reshape(NS, S)      # (NS, S)
        own = np.repeat(np.arange(BL), M)                # (NS,)
        nlow = (blk < VSPLIT).sum(1)
        order = np.argsort(nlow, kind="stable")
        st_c, nl_c, ow_c = [], [], []
        for q in ((0, 3, 1, 2) if NG == 4 else range(NG)):
            pick = order[q * P:(q + 1) * P]
            st_c.append(np.sort(blk[pick], axis=1))
            nl_c.append(nlow[pick])
            ow_c.append(own[pick])
        sorted_toks.append(st_c)
        nlows.append(nl_c)
        owners.append(ow_c)

    KA = tuple(int(max(nlows[i][g].max() for i in range(NCORES))) for g in range(NG))
    KB = tuple(S - int(min(nlows[i][g].min() for i in range(NCORES))) for g in range(NG))

    consts = _consts()
    in_maps = []
    for i in range(NCORES):
        m = {"ccat": ccat}
        m["identg"] = consts["identg"]
        sel = np.zeros((P, NG * 2), np.float32)
        mneg = np.full((BL, BL * M), NEG, np.float32)
        for g in range(NG):
            sel[np.arange(P), g * 2 + owners[i][g]] = 1.0
            mneg[owners[i][g], g * P + np.arange(P)] = 0.0
        cst = np.zeros((P, P + NG * 2 + 2), np.float32)
        cst[:, 0:P] = consts["ident"]
        cst[:, P:P + NG * 2] = sel
        cst[0:2, P + NG * 2:P + NG * 2 + 2] = consts["i2"]
        m["cst"] = cst
        m["mneg"] = mneg
        idx_cols = []
        for g in range(NG):
            toks = sorted_toks[i][g]                     # (P, S) sorted
            nlow = nlows[i][g]                           # (P,)
            ka, kb = KA[g], KB[g]
            ks = np.arange(ka)[:, None]                  # slot k -> row k*128+p
            low = np.where(ks < nlow[None, :], toks.T[:ka], 0).astype(np.int16)
            idx_cols.append(_wrap16(low.reshape(-1)))
            k0 = S - kb
            ksb = (k0 + np.arange(kb))[:, None]
            high = np.where(
                ksb >= nlow[None, :],
                toks.T[k0:].astype(np.int64) - VSPLIT,
                ZHIGH,
            ).astype(np.int16)
            idx_cols.append(_wrap16(high.reshape(-1)))
        m["idx"] = np.concatenate(idx_cols, axis=1)
        in_maps.append(m)
    return in_maps, KA, KB


def run(in_maps, KA, KB, trace=False, **kwargs):
    from concourse.bass_utils import run_bass_kernel_spmd

    key = ("v2", KA, KB)
    if key not in _CACHE:
        _CACHE[key] = build(KA, KB)
    nc = _CACHE[key]
    res = run_bass_kernel_spmd(
        nc, in_maps, core_ids=list(range(NCORES)), trace=trace, **kwargs
    )
    out = np.concatenate([r["out"] for r in res.results], axis=0)
    return out, res


def kernel(story, C):
    in_maps, KA, KB = prep_inputs(story, C)
    out, _ = run(in_maps, KA, KB)
    return out.astype(np.float32)


# revision 4
# speedup vs baseline: 1.4305x; 1.0540x over previous
"""Trainium2 Bass kernel for nn_EncoderMemNN_14929306321427 (MemNN encoder).

Math (see reference.py): story (M=256, B=16, S=64) token ids; C (4, V, 128)
embedding tables. Per hop h: m_A = sum_S C[h][s], prob = softmax_M(m_A @ u),
m_C = sum_S C[h+1][s], u += prob @ m_C. u starts at 0, so hop-0's softmax is
uniform: C[0] is never needed and u after hop 0 is mean_M(E1).

Strategy: data-parallel over batch (2 rows/core, 8 cores, no collectives).
Host fuses tables 1..3 into ccat[V+1, 384] fp16 (row V = 0) so each token is
ONE 768B dma_gather row. dma_gather indices are int16, so tokens are split at
32768: low tokens gather from the table base, high tokens from a +32768 row
view; slots not owned by a call point at an all-zero row (PAD row 0 /
appended row V), which adds 0 to the sum. Tokens are sorted within each
sentence and sentences are nlow-balanced across groups so the two sections
cover disjoint near-minimal slot ranges (~10% filler). The sentence-sum runs
on the PE as identity-matmul accumulation into PSUM (fp32-exact), then a tiny
PE/ACT/DVE attention pipeline computes the 3 hops.

Perf: SWDGE descriptor generation is the bottleneck (~7.9 ns/desc), and each
SWDGE queue is served by a single pair of the 8 Q7 cores (q7 kernel:
cpu_id/2 == queue_num). So each group's gather is split into ~equal slot
ranges across all 4 SWDGE queues, engaging all 8 Q7 cores in parallel.
Queue-0 gathers block the Pool engine for their whole generation (observed;
queues 1-3 don't), so each group's queue-0 call is dispatched last.
"""

import numpy as np

HOPS = 3
V = 50257
D = 128
M = 256
B = 16
S = 64
NCORES = 8
BL = B // NCORES            # batch rows per core
NS = BL * M                 # sentences per core
P = 128
NG = NS // P                # sentence groups of 128
DCAT = HOPS * D             # 384 = fused row [C1|C2|C3]
NEG = -1e30
VSPLIT = 32768              # A window: rows [0, 32768)
BBASE = V + 1 - 32768       # B window: rows [BBASE, V]; idx 32767 -> zero row V
NQ = 4                      # SWDGE queues (all 8 Q7 cores)

_CACHE = {}


def _consts():
    ident = np.eye(P, dtype=np.float32)
    i2 = np.eye(2, dtype=np.float32)
    identg = np.eye(P, dtype=np.float16)
    return {"ident": ident, "i2": i2, "identg": identg}


def _plan(ka, kb):
    """Split the concatenated [A slots | B slots] list of one group into NQ
    near-equal contiguous ranges, one per SWDGE queue; a range crossing the
    A/B boundary becomes two calls on the same queue. Queue 0 is dispatched
    last (its generation blocks the Pool engine)."""
    tot = ka + kb
    bounds = [round(tot * i / NQ) for i in range(NQ + 1)]
    calls = []                       # (section, slot_off, slot_cnt, queue)
    for i in range(NQ):
        q = (i + 1) % NQ             # ranges -> queues 1,2,3,0
        s, e = bounds[i], bounds[i + 1]
        if s < ka and min(e, ka) > s:
            calls.append(("a", s, min(e, ka) - s, q))
        if e > ka and e > max(s, ka):
            calls.append(("b", max(s, ka) - ka, e - max(s, ka), q))
    calls.sort(key=lambda c: c[3] == 0)   # stable: queues 1,2,3 first, 0 last
    return calls


def build(KA, KB, do_compile=True):
    """KA/KB: per-group slot counts for the low/high gather sections."""
    from concourse import bacc, mybir, tile

    f32 = mybir.dt.float32
    f16 = mybir.dt.float16
    i16 = mybir.dt.int16
    Alu = mybir.AluOpType
    Act = mybir.ActivationFunctionType
    Ax = mybir.AxisListType

    nc = bacc.Bacc(num_swdge_queues=NQ)
    ccat_d = nc.declare_dram_parameter("ccat", [V + 1, DCAT], f16, isOutput=False)
    # per-group wrapped idx arrays: [P, 8*(KA[g]+KB[g])] each (A cols then B)
    idx_d = [
        nc.declare_dram_parameter(f"idx{g}", [P, 8 * (KA[g] + KB[g])], i16,
                                  isOutput=False)
        for g in range(NG)
    ]
    # fused f32 consts: [ident | sel | i2pad] = [P, P + NG*2 + 2]
    cst_d = nc.declare_dram_parameter("cst", [P, P + NG * 2 + 2], f32, isOutput=False)
    identg_d = nc.declare_dram_parameter("identg", [P, P], f16, isOutput=False)
    mneg_d = nc.declare_dram_parameter("mneg", [BL, BL * M], f32, isOutput=False)
    out_d = nc.declare_dram_parameter("out", [BL, D], f32, isOutput=True)

    with tile.TileContext(nc) as tc:
        with (
            tc.tile_pool(name="const", bufs=1) as cpool,
            tc.tile_pool(name="gather", bufs=2) as gpool,
            tc.tile_pool(name="work", bufs=2) as wpool,
            tc.tile_pool(name="ps_e", bufs=2, space="PSUM") as ps_e,
            tc.tile_pool(name="ps_t", bufs=2, space="PSUM") as ps_t,
            tc.tile_pool(name="ps_col", bufs=1, space="PSUM") as ps_col,
            tc.tile_pool(name="ps_mm", bufs=1, space="PSUM") as ps_mm,
        ):
            idx_sb = []
            for g in range(NG):
                t = cpool.tile([P, 8 * (KA[g] + KB[g])], i16, tag=f"idx{g}")
                nc.sync.dma_start(out=t[:], in_=idx_d[g][:])
                idx_sb.append(t)
            cst = cpool.tile([P, P + NG * 2 + 2], f32)
            nc.sync.dma_start(out=cst[:], in_=cst_d[:])
            identg = cpool.tile([P, P], f16)
            nc.sync.dma_start(out=identg[:], in_=identg_d[:])
            mneg = cpool.tile([BL, BL * M], f32)
            nc.sync.dma_start(out=mneg[:], in_=mneg_d[:])
            ident = cst[:, 0:P]
            sel = cst[:, P:P + NG * 2]
            i2 = cst[0:2, P + NG * 2:P + NG * 2 + 2]

            # ---- gather + sentence-sum: E_all[p, g*DCAT+d] = sum_S ccat[tok]
            E_all = cpool.tile([P, NG * DCAT], f32)
            for g in range(NG):
                tiles = []
                for j, (sec, soff, cnt, q) in enumerate(_plan(KA[g], KB[g])):
                    ct = gpool.tile([P, cnt, DCAT], f16, tag=f"gt{j}")
                    src = ccat_d[:] if sec == "a" else ccat_d[BBASE:, :]
                    col = (0 if sec == "a" else 8 * KA[g]) + soff * 8
                    nc.gpsimd.dma_gather(
                        out_ap=ct[:], in_ap=src,
                        idxs_ap=idx_sb[g][:, col:col + cnt * 8],
                        num_idxs=P * cnt, num_idxs_reg=P * cnt,
                        elem_size=DCAT, single_packet=False, queue_num=q,
                    )
                    tiles.append((ct, cnt))
                eps = ps_e.tile([P, DCAT], f32, tag="eacc")
                tot = KA[g] + KB[g]
                nmm = 0
                for ct, cnt in tiles:
                    for r in range(cnt):
                        nc.tensor.matmul(
                            out=eps[:], lhsT=identg[:], rhs=ct[:, r, :],
                            start=(nmm == 0), stop=(nmm == tot - 1),
                        )
                        nmm += 1
                nc.vector.tensor_copy(out=E_all[:, g * DCAT:(g + 1) * DCAT], in_=eps[:])

            # transposed E1/E2 for the logits matmuls (filled per group):
            # F_t[:, g*P:(g+1)*P] = (E_t block of group g).T   [d, sentence]
            F1 = cpool.tile([P, NS], f32)
            F2 = cpool.tile([P, NS], f32)
            us = ps_mm.tile([BL, DCAT], f32, tag="usum")
            for g in range(NG):
                for t, F in ((0, F1), (1, F2)):
                    tp = ps_t.tile([P, P], f32, tag="tp")
                    nc.tensor.transpose(
                        out=tp[:],
                        in_=E_all[:, g * DCAT + t * D: g * DCAT + t * D + D],
                        identity=ident,
                    )
                    nc.scalar.copy(out=F[:, g * P:(g + 1) * P], in_=tp[:])
                # hop 0: u = mean_M E1[b] (softmax of zero logits is uniform);
                # sel col b marks this group's sentences owned by batch row b
                nc.tensor.matmul(
                    out=us[:], lhsT=sel[:, g * 2:(g + 1) * 2],
                    rhs=E_all[:, g * DCAT:(g + 1) * DCAT],
                    start=(g == 0), stop=(g == NG - 1),
                )
            u = wpool.tile([BL, D], f32, tag="u0")
            nc.scalar.activation(
                out=u[:], in_=us[0:BL, 0:D], func=Act.Copy, scale=1.0 / M
            )

            # ---- hops 1..2
            for hop in (1, 2):
                F = F1 if hop == 1 else F2
                # u as columns: uc[d, b] = u[b, d]
                uc_ps = ps_col.tile([P, BL], f32, tag="colT")
                nc.tensor.matmul(out=uc_ps[:], lhsT=u[:], rhs=i2, start=True, stop=True)
                uc = wpool.tile([P, BL], f32, tag="uc")
                nc.scalar.copy(out=uc[:], in_=uc_ps[:])
                # logits[b, b'*M+m] = sum_d u[b,d] * E_hop[b',m,d]; mask kills b!=b'
                lg_ps = ps_mm.tile([BL, NS], f32, tag="lg")
                nc.tensor.matmul(out=lg_ps[:], lhsT=uc[:], rhs=F[:], start=True, stop=True)
                lgm = wpool.tile([BL, NS], f32, tag="lgm")
                nc.vector.scalar_tensor_tensor(
                    out=lgm[:], in0=lg_ps[:], scalar=1.0, in1=mneg[:],
                    op0=Alu.mult, op1=Alu.add,
                )
                nmax = wpool.tile([BL, 1], f32, tag="nmax")
                nc.vector.tensor_reduce(
                    out=nmax[:], in_=lgm[:], axis=Ax.X, op=Alu.max, negate=True
                )
                pe = wpool.tile([BL, NS], f32, tag="pe")
                den = wpool.tile([BL, 1], f32, tag="den")
                nc.scalar.activation(
                    out=pe[:], in_=lgm[:], func=Act.Exp, bias=nmax[:], scale=1.0,
                    accum_out=den[:],
                )
                rden = wpool.tile([BL, 1], f32, tag="rden")
                nc.vector.reciprocal(out=rden[:], in_=den[:])
                # o[b, d] = sum_m pe[b, m] * E_{hop+1}[b, m, d]  (normalized below)
                o_ps = ps_mm.tile([BL, D], f32, tag="o")
                for g in range(NG):
                    pt_ps = ps_col.tile([P, BL], f32, tag="colT")
                    nc.tensor.matmul(
                        out=pt_ps[:], lhsT=pe[:, g * P:(g + 1) * P], rhs=i2,
                        start=True, stop=True,
                    )
                    ptsb = wpool.tile([P, BL], f32, tag="ptsb")
                    nc.scalar.copy(out=ptsb[:], in_=pt_ps[:])
                    nc.tensor.matmul(
                        out=o_ps[:], lhsT=ptsb[:],
                        rhs=E_all[:, g * DCAT + hop * D: g * DCAT + hop * D + D],
                        start=(g == 0), stop=(g == NG - 1),
                    )
                # u <- u + o / den
                u2 = wpool.tile([BL, D], f32, tag=f"u{hop}")
                nc.vector.scalar_tensor_tensor(
                    out=u2[:], in0=o_ps[:], scalar=rden[:], in1=u[:],
                    op0=Alu.mult, op1=Alu.add,
                )
                u = u2

            nc.sync.dma_start(out=out_d[:], in_=u[:])
    if do_compile:
        nc.compile()
    return nc


def _wrap16(idx):
    """flat [n] int16 -> SBUF layout [128, n//16]: value i at [i%16, i//16],
    replicated to the 8 16-partition groups the Q7 cores read."""
    n = idx.shape[0]
    w = np.zeros((16, n // 16), np.int16)
    w[np.arange(n) % 16, np.arange(n) // 16] = idx
    return np.tile(w, (8, 1))


def prep_inputs(story, C):
    """Host-side: fused fp16 table, sorted/balanced per-core index layouts."""
    story = np.asarray(story)
    C = np.asarray(C, dtype=np.float32)
    s = story.transpose(1, 0, 2).astype(np.int32)       # (B, M, S)
    ccat = np.zeros((V + 1, DCAT), np.float16)
    ccat[:V] = np.concatenate([C[1], C[2], C[3]], axis=1).astype(np.float16)

    # per core: sort tokens in each sentence; sentence's first sA tokens go to
    # the A window (rows < 32768), the rest to the overlapping B window (rows
    # >= BBASE). Sentences are grouped by nA (count of tokens below BBASE,
    # which MUST go to A); the mid band [BBASE, 32768) can go to either side,
    # so nearly every group splits exactly 64 = KA + KB with zero filler.
    sorted_toks = []                                     # [core][g] -> (P, S)
    sAs = []                                             # [core][g] -> (P,)
    owners = []                                          # [core][g] -> (P,) batch row
    picks = []                                           # [core][g] -> (P,) nA,nlow
    for i in range(NCORES):
        blk = s[i * BL:(i + 1) * BL].reshape(NS, S)      # (NS, S)
        own = np.repeat(np.arange(BL), M)                # (NS,)
        st = np.sort(blk, axis=1)
        nA = (st < BBASE).sum(1)
        nlow = (st < VSPLIT).sum(1)
        order = np.argsort(nA, kind="stable")
        st_c, ow_c, pk_c = [], [], []
        for g in range(NG):
            pick = order[g * P:(g + 1) * P]
            st_c.append(st[pick])
            ow_c.append(own[pick])
            pk_c.append((nA[pick], nlow[pick]))
        sorted_toks.append(st_c)
        owners.append(ow_c)
        picks.append(pk_c)

    KA, KB, sAs = [], [], []
    for g in range(NG):
        ka = int(max(picks[i][g][0].max() for i in range(NCORES)))
        KA.append(ka)
        sA_c = [np.minimum(ka, picks[i][g][1]) for i in range(NCORES)]
        KB.append(int(max(S - sA_c[i].min() for i in range(NCORES))))
        sAs.append(sA_c)
    KA, KB = tuple(KA), tuple(KB)
    consts = _consts()
    in_maps = []
    for i in range(NCORES):
        m = {"ccat": ccat}
        m["identg"] = consts["identg"]
        sel = np.zeros((P, NG * 2), np.float32)
        mneg = np.full((BL, BL * M), NEG, np.float32)
        for g in range(NG):
            sel[np.arange(P), g * 2 + owners[i][g]] = 1.0
            mneg[owners[i][g], g * P + np.arange(P)] = 0.0
        cst = np.zeros((P, P + NG * 2 + 2), np.float32)
        cst[:, 0:P] = consts["ident"]
        cst[:, P:P + NG * 2] = sel
        cst[0:2, P + NG * 2:P + NG * 2 + 2] = consts["i2"]
        m["cst"] = cst
        m["mneg"] = mneg
        for g in range(NG):
            toks = sorted_toks[i][g]                     # (P, S) sorted
            sA = sAs[g][i]                               # (P,) tokens sent to A
            ka, kb = KA[g], KB[g]
            ks = np.arange(ka)[:, None]                  # A slot k -> token rank k
            low = np.where(ks < sA[None, :], toks.T[:ka], 0).astype(np.int16)
            js = np.arange(kb)[:, None]                  # B slot j -> rank sA + j
            rows = np.minimum(sA[None, :] + js, S - 1)
            high = np.where(
                js < (S - sA)[None, :],
                np.take_along_axis(toks.T, rows, axis=0).astype(np.int64) - BBASE,
                32767,
            ).astype(np.int16)
            m[f"idx{g}"] = np.concatenate(
                [_wrap16(low.reshape(-1)), _wrap16(high.reshape(-1))], axis=1)
        in_maps.append(m)
    return in_maps, KA, KB


def run(in_maps, KA, KB, trace=False, **kwargs):
    from concourse.bass_utils import run_bass_kernel_spmd

    key = ("v3", KA, KB)
    if key not in _CACHE:
        _CACHE[key] = build(KA, KB)
    nc = _CACHE[key]
    res = run_bass_kernel_spmd(
        nc, in_maps, core_ids=list(range(NCORES)), trace=trace, **kwargs
    )
    out = np.concatenate([r["out"] for r in res.results], axis=0)
    return out, res


def kernel(story, C):
    in_maps, KA, KB = prep_inputs(story, C)
    out, _ = run(in_maps, KA, KB)
    return out.astype(np.float32)


# revision 6
# speedup vs baseline: 1.8133x; 1.2676x over previous
"""Trainium2 Bass kernel for nn_EncoderMemNN_14929306321427 (MemNN encoder).

Math (see reference.py): story (M=256, B=16, S=64) token ids; C (4, V, 128)
embedding tables. Per hop h: m_A = sum_S C[h][s], prob = softmax_M(m_A @ u),
m_C = sum_S C[h+1][s], u += prob @ m_C. u starts at 0, so hop-0's softmax is
uniform: C[0] is never needed and u after hop 0 is mean_M(E1).

Strategy: data-parallel over batch (2 rows/core, 8 cores, no collectives).
Host fuses tables 1..3 into ccat[V+1, 384] fp16 (row V = 0) so each token is
ONE 768B dma_gather row. dma_gather indices are int16, so tokens are split at
32768: low tokens gather from the table base, high tokens from a +32768 row
view; slots not owned by a call point at an all-zero row (PAD row 0 /
appended row V), which adds 0 to the sum. Tokens are sorted within each
sentence and sentences are nlow-balanced across groups so the two sections
cover disjoint near-minimal slot ranges (~10% filler). The sentence-sum runs
on the PE as identity-matmul accumulation into PSUM (fp32-exact), then a tiny
PE/ACT/DVE attention pipeline computes the 3 hops.

Perf: SWDGE descriptor generation is the bottleneck (~7.9 ns/desc), and each
SWDGE queue is served by a single pair of the 8 Q7 cores (q7 kernel:
cpu_id/2 == queue_num). So each group's gather is split into ~equal slot
ranges across all 4 SWDGE queues, engaging all 8 Q7 cores in parallel.
Queue-0 gathers block the Pool engine for their whole generation (observed;
queues 1-3 don't), so each group's queue-0 call is dispatched last.
"""

import numpy as np

HOPS = 3
V = 50257
D = 128
M = 256
B = 16
S = 64
NCORES = 8
BL = B // NCORES            # batch rows per core
NS = BL * M                 # sentences per core
P = 128
NG = NS // P                # sentence groups of 128
DCAT = HOPS * D             # 384 = fused row [C1|C2|C3]
NEG = -1e30
VSPLIT = 32768              # A window: rows [0, 32768)
BBASE = V + 1 - 32768       # B window: rows [BBASE, V]; idx 32767 -> zero row V
NQ = 4                      # SWDGE queues (all 8 Q7 cores)

_CACHE = {}


def _consts():
    ident = np.eye(P, dtype=np.float32)
    i2 = np.eye(2, dtype=np.float32)
    identg = np.eye(P, dtype=np.float16)
    return {"ident": ident, "i2": i2, "identg": identg}


def _plan(ka, kb):
    """Split the concatenated [A slots | B slots] list of one group into NQ
    near-equal contiguous ranges, one per SWDGE queue; a range crossing the
    A/B boundary becomes two calls on the same queue. Queue 0 is dispatched
    last (its generation blocks the Pool engine)."""
    tot = ka + kb
    bounds = [round(tot * i / NQ) for i in range(NQ + 1)]
    calls = []                       # (section, slot_off, slot_cnt, queue)
    for i in range(NQ):
        q = (i + 1) % NQ             # ranges -> queues 1,2,3,0
        s, e = bounds[i], bounds[i + 1]
        if s < ka and min(e, ka) > s:
            calls.append(("a", s, min(e, ka) - s, q))
        if e > ka and e > max(s, ka):
            calls.append(("b", max(s, ka) - ka, e - max(s, ka), q))
    calls.sort(key=lambda c: c[3] == 0)   # stable: queues 1,2,3 first, 0 last
    return calls


def build(KA, KB, do_compile=True):
    """KA/KB: per-group slot counts for the low/high gather sections."""
    from concourse import bacc, mybir, tile

    f32 = mybir.dt.float32
    f16 = mybir.dt.float16
    i16 = mybir.dt.int16
    Alu = mybir.AluOpType
    Act = mybir.ActivationFunctionType
    Ax = mybir.AxisListType

    nc = bacc.Bacc(num_swdge_queues=NQ)
    ccat_d = nc.declare_dram_parameter("ccat", [V + 1, DCAT], f16, isOutput=False)
    # per-group wrapped idx arrays: [P, 8*(KA[g]+KB[g])] each (A cols then B)
    idx_d = [
        nc.declare_dram_parameter(f"idx{g}", [P, 8 * (KA[g] + KB[g])], i16,
                                  isOutput=False)
        for g in range(NG)
    ]
    # fused f32 consts: [ident | sel | ones] = [P, P + NG*2 + 128]
    cst_d = nc.declare_dram_parameter("cst", [P, P + NG * 2 + 128], f32, isOutput=False)
    identg_d = nc.declare_dram_parameter("identg", [P, P], f16, isOutput=False)
    out_d = nc.declare_dram_parameter("out", [BL, D], f32, isOutput=True)

    with tile.TileContext(nc) as tc:
        with (
            tc.tile_pool(name="const", bufs=1) as cpool,
            tc.tile_pool(name="gather", bufs=2) as gpool,
            tc.tile_pool(name="work", bufs=2) as wpool,
            tc.tile_pool(name="ps_e", bufs=2, space="PSUM") as ps_e,
            tc.tile_pool(name="ps_t", bufs=2, space="PSUM") as ps_t,
            tc.tile_pool(name="ps_us", bufs=1, space="PSUM") as ps_us,
            tc.tile_pool(name="ps_o", bufs=1, space="PSUM") as ps_o,
            tc.tile_pool(name="ps_den", bufs=1, space="PSUM") as ps_den,
        ):
            idx_sb = []
            for g in range(NG):
                t = cpool.tile([P, 8 * (KA[g] + KB[g])], i16, tag=f"idx{g}")
                nc.sync.dma_start(out=t[:], in_=idx_d[g][:])
                idx_sb.append(t)
            cst = cpool.tile([P, P + NG * 2 + 128], f32)
            nc.sync.dma_start(out=cst[:], in_=cst_d[:])
            identg = cpool.tile([P, P], f16)
            nc.sync.dma_start(out=identg[:], in_=identg_d[:])
            ident = cst[:, 0:P]
            sel = cst[:, P:P + NG * 2]
            onescol = cst[:, P + NG * 2:P + NG * 2 + 1]
            onesrow = cst[0:1, P + NG * 2:P + NG * 2 + 128]

            # ---- gather + sentence-sum: E_all[p, g*DCAT+d] = sum_S ccat[tok]
            E_all = cpool.tile([P, NG * DCAT], f32)
            for g in range(NG):
                tiles = []
                for j, (sec, soff, cnt, q) in enumerate(_plan(KA[g], KB[g])):
                    ct = gpool.tile([P, cnt, DCAT], f16, tag=f"gt{j}")
                    src = ccat_d[:] if sec == "a" else ccat_d[BBASE:, :]
                    col = (0 if sec == "a" else 8 * KA[g]) + soff * 8
                    nc.gpsimd.dma_gather(
                        out_ap=ct[:], in_ap=src,
                        idxs_ap=idx_sb[g][:, col:col + cnt * 8],
                        num_idxs=P * cnt, num_idxs_reg=P * cnt,
                        elem_size=DCAT, single_packet=False, queue_num=q,
                    )
                    tiles.append((ct, cnt))
                eps = ps_e.tile([P, DCAT], f32, tag="eacc")
                tot = KA[g] + KB[g]
                nmm = 0
                for ct, cnt in tiles:
                    for r in range(cnt):
                        nc.tensor.matmul(
                            out=eps[:], lhsT=identg[:], rhs=ct[:, r, :],
                            start=(nmm == 0), stop=(nmm == tot - 1),
                        )
                        nmm += 1
                nc.vector.tensor_copy(out=E_all[:, g * DCAT:(g + 1) * DCAT], in_=eps[:])

            # transposed E1/E2 for the logits matmuls (filled per group):
            # F_t[:, g*P:(g+1)*P] = (E_t block of group g).T   [d, sentence]
            # usT[d, b] = sum over group-g sentences owned by b of E1 (hop 0:
            # softmax of zero logits is uniform, so u0 = mean_M E1).
            F1 = cpool.tile([P, NS], f32)
            F2 = cpool.tile([P, NS], f32)
            usT = ps_us.tile([P, BL], f32, tag="usT")
            for g in range(NG):
                for t, F in ((0, F1), (1, F2)):
                    tp = ps_t.tile([P, P], f32, tag="tp")
                    nc.tensor.transpose(
                        out=tp[:],
                        in_=E_all[:, g * DCAT + t * D: g * DCAT + t * D + D],
                        identity=ident,
                    )
                    nc.scalar.copy(out=F[:, g * P:(g + 1) * P], in_=tp[:])
                nc.tensor.matmul(
                    out=usT[:], lhsT=E_all[:, g * DCAT:g * DCAT + D],
                    rhs=sel[:, g * 2:(g + 1) * 2],
                    start=(g == 0), stop=(g == NG - 1),
                )
            uc = wpool.tile([P, BL], f32, tag="uc0")
            nc.scalar.activation(
                out=uc[:], in_=usT[:], func=Act.Copy, scale=1.0 / M
            )

            # ---- hops 1..2, fully in transposed [d|m, b] layout.
            # lgT_g = F_g.T @ uc; peT = exp(lgT) (logits are O(5), no max
            # needed in fp32); mask other-batch sentences by multiplying with
            # sel; oT += E_g.T @ peTm; den += ones.T @ peTm; u += oT/den.
            for hop in (1, 2):
                F = F1 if hop == 1 else F2
                pes = []
                for g in range(NG):
                    lgT = ps_t.tile([P, BL], f32, tag="tp")
                    nc.tensor.matmul(
                        out=lgT[:], lhsT=F[:, g * P:(g + 1) * P], rhs=uc[:],
                        start=True, stop=True,
                    )
                    peT = wpool.tile([P, BL], f32, tag=f"peT{g}")
                    nc.scalar.activation(
                        out=peT[:], in_=lgT[:], func=Act.Exp, scale=1.0
                    )
                    peTm = wpool.tile([P, BL], f32, tag=f"peTm{g}")
                    nc.vector.scalar_tensor_tensor(
                        out=peTm[:], in0=peT[:], scalar=1.0,
                        in1=sel[:, g * 2:(g + 1) * 2],
                        op0=Alu.mult, op1=Alu.mult,
                    )
                    pes.append(peTm)
                oT = ps_o.tile([P, BL], f32, tag="oT")
                den = ps_den.tile([1, BL], f32, tag="den")
                for g in range(NG):
                    nc.tensor.matmul(
                        out=oT[:],
                        lhsT=E_all[:, g * DCAT + hop * D: g * DCAT + hop * D + D],
                        rhs=pes[g][:], start=(g == 0), stop=(g == NG - 1),
                    )
                    nc.tensor.matmul(
                        out=den[:], lhsT=onescol, rhs=pes[g][:],
                        start=(g == 0), stop=(g == NG - 1),
                    )
                rden = wpool.tile([1, BL], f32, tag="rden")
                nc.vector.reciprocal(out=rden[:], in_=den[:])
                rb_ps = ps_t.tile([P, BL], f32, tag="tp")
                nc.tensor.matmul(
                    out=rb_ps[:], lhsT=onesrow, rhs=rden[:], start=True, stop=True
                )
                rb = wpool.tile([P, BL], f32, tag="rb")
                nc.scalar.copy(out=rb[:], in_=rb_ps[:])
                t1 = wpool.tile([P, BL], f32, tag="t1")
                nc.vector.scalar_tensor_tensor(
                    out=t1[:], in0=oT[:], scalar=1.0, in1=rb[:],
                    op0=Alu.mult, op1=Alu.mult,
                )
                u2 = wpool.tile([P, BL], f32, tag=f"u{hop}c")
                nc.vector.scalar_tensor_tensor(
                    out=u2[:], in0=t1[:], scalar=1.0, in1=uc[:],
                    op0=Alu.mult, op1=Alu.add,
                )
                uc = u2

            # out[b, d] = uc[d, b]
            fin = ps_t.tile([P, P], f32, tag="tp")
            nc.tensor.matmul(
                out=fin[0:BL, :], lhsT=uc[:], rhs=ident, start=True, stop=True
            )
            uo = wpool.tile([BL, D], f32, tag="uo")
            nc.scalar.copy(out=uo[:], in_=fin[0:BL, :])
            nc.sync.dma_start(out=out_d[:], in_=uo[:])
    if do_compile:
        nc.compile()
    return nc


def _wrap16(idx):
    """flat [n] int16 -> SBUF layout [128, n//16]: value i at [i%16, i//16],
    replicated to the 8 16-partition groups the Q7 cores read."""
    n = idx.shape[0]
    w = np.zeros((16, n // 16), np.int16)
    w[np.arange(n) % 16, np.arange(n) // 16] = idx
    return np.tile(w, (8, 1))


def prep_inputs(story, C):
    """Host-side: fused fp16 table, sorted/balanced per-core index layouts."""
    story = np.asarray(story)
    C = np.asarray(C, dtype=np.float32)
    s = story.transpose(1, 0, 2).astype(np.int32)       # (B, M, S)
    ccat = np.zeros((V + 1, DCAT), np.float16)
    ccat[:V] = np.concatenate([C[1], C[2], C[3]], axis=1).astype(np.float16)

    # per core: sort tokens in each sentence; sentence's first sA tokens go to
    # the A window (rows < 32768), the rest to the overlapping B window (rows
    # >= BBASE). Sentences are grouped by nA (count of tokens below BBASE,
    # which MUST go to A); the mid band [BBASE, 32768) can go to either side,
    # so nearly every group splits exactly 64 = KA + KB with zero filler.
    sorted_toks = []                                     # [core][g] -> (P, S)
    sAs = []                                             # [core][g] -> (P,)
    owners = []                                          # [core][g] -> (P,) batch row
    picks = []                                           # [core][g] -> (P,) nA,nlow
    for i in range(NCORES):
        blk = s[i * BL:(i + 1) * BL].reshape(NS, S)      # (NS, S)
        own = np.repeat(np.arange(BL), M)                # (NS,)
        st = np.sort(blk, axis=1)
        nA = (st < BBASE).sum(1)
        nlow = (st < VSPLIT).sum(1)
        order = np.argsort(nA, kind="stable")
        st_c, ow_c, pk_c = [], [], []
        for g in range(NG):
            pick = order[g * P:(g + 1) * P]
            st_c.append(st[pick])
            ow_c.append(own[pick])
            pk_c.append((nA[pick], nlow[pick]))
        sorted_toks.append(st_c)
        owners.append(ow_c)
        picks.append(pk_c)

    KA, KB, sAs = [], [], []
    for g in range(NG):
        ka = int(max(picks[i][g][0].max() for i in range(NCORES)))
        KA.append(ka)
        sA_c = [np.minimum(ka, picks[i][g][1]) for i in range(NCORES)]
        KB.append(int(max(S - sA_c[i].min() for i in range(NCORES))))
        sAs.append(sA_c)
    KA, KB = tuple(KA), tuple(KB)
    consts = _consts()
    in_maps = []
    for i in range(NCORES):
        m = {"ccat": ccat}
        m["identg"] = consts["identg"]
        sel = np.zeros((P, NG * 2), np.float32)
        for g in range(NG):
            sel[np.arange(P), g * 2 + owners[i][g]] = 1.0
        cst = np.zeros((P, P + NG * 2 + 128), np.float32)
        cst[:, 0:P] = consts["ident"]
        cst[:, P:P + NG * 2] = sel
        cst[:, P + NG * 2:] = 1.0
        m["cst"] = cst
        for g in range(NG):
            toks = sorted_toks[i][g]                     # (P, S) sorted
            sA = sAs[g][i]                               # (P,) tokens sent to A
            ka, kb = KA[g], KB[g]
            ks = np.arange(ka)[:, None]                  # A slot k -> token rank k
            low = np.where(ks < sA[None, :], toks.T[:ka], 0).astype(np.int16)
            js = np.arange(kb)[:, None]                  # B slot j -> rank sA + j
            rows = np.minimum(sA[None, :] + js, S - 1)
            high = np.where(
                js < (S - sA)[None, :],
                np.take_along_axis(toks.T, rows, axis=0).astype(np.int64) - BBASE,
                32767,
            ).astype(np.int16)
            m[f"idx{g}"] = np.concatenate(
                [_wrap16(low.reshape(-1)), _wrap16(high.reshape(-1))], axis=1)
        in_maps.append(m)
    return in_maps, KA, KB


def run(in_maps, KA, KB, trace=False, **kwargs):
    from concourse.bass_utils import run_bass_kernel_spmd

    key = ("v4", KA, KB)
    if key not in _CACHE:
        _CACHE[key] = build(KA, KB)
    nc = _CACHE[key]
    res = run_bass_kernel_spmd(
        nc, in_maps, core_ids=list(range(NCORES)), trace=trace, **kwargs
    )
    out = np.concatenate([r["out"] for r in res.results], axis=0)
    return out, res


def kernel(story, C):
    in_maps, KA, KB = prep_inputs(story, C)
    out, _ = run(in_maps, KA, KB)
    return out.astype(np.float32)
